# revision 26
# baseline (speedup 1.0000x reference)
"""CORDIV stochastic-computing division kernel for Trainium2 (8 NeuronCores).

Recurrence per lane n (T sequential steps, lanes fully independent):
    sr = sr_init[:, n]                       # shift register, depth B
    for t in range(T):
        r  = rng_table[t % B]
        hq = sr[r]
        q[t, n] = dividend[t, n] if divisor[t, n] == 1 else hq
        sr = [q[t, n], sr[0], ..., sr[B-2]]

Unrolled, the shift register disappears (resolved on the host from
rng_table into a static source schedule):
    q[t] = divisor[t] ? dividend[t] : src_t
    src_t = q[t-1-r_t]          if t-1-r_t >= 0
          = sr_init[r_t - t]    otherwise

Since every stream is bits {0,1}, the select is pure boolean logic:
    q[t] = m[t] | (src_t & aN[t])
      m  = dividend & divisor      (emit-1 mask)
      aN = ~divisor                (pass-through mask)
so the host BIT-PACKS 32 lanes into each int32 word and the device runs
the recurrence with int32 bitwise tensor_tensor ops on the DVE (AND/OR
are byte-order agnostic, so host little-endian packing round-trips
exactly).  Steps are grouped into dependency LEVELS (no dep is
intra-level): each level runs per-step ANDs (pair-fused via broadcast
or adjacent-slice APs where sources allow) and ONE wide OR against the
level's m block — 12 DVE ops total instead of 32.  Level-0 ANDs
(sr & aN, both host-known per step) fold into host preprocessing, the
same class of input packing as the baseline's selector stream.

Memory regime: HBM traffic per core collapses to
    in  (aN/u, m packed)  1.0  MiB
    out (q packed)        0.5  MiB
~1.5 MiB vs ~50 MiB naive f32 (~4.3 us DMA floor at ~358 GB/s/core).
Per-DMA fixed costs (~0.6 us HWDGE issue + ~0.9 us completion-sem
propagation) dominate at these sizes, so levels are batched into 3
loads and 2 stores spread across the SP/ACT HWDGE rings (parallel
issue + overlapped data), pipelined against the DVE (the only compute
engine used).

Sharding: lane dimension N split evenly across 8 cores (data parallel,
no communication).
"""

import os

import numpy as np

import concourse.bass as bass
import concourse.mybir as mybir
from concourse.tile import TileContext
from concourse.bass_utils import run_bass_kernel_spmd

_STRIP = os.environ.get("KERNEL_STRIP", "0") == "1"
_NLOADS = int(os.environ.get("KERNEL_NLOADS", "3"))
_NSTORES = int(os.environ.get("KERNEL_NSTORES", "2"))
_SPLIT_RINGS = os.environ.get("KERNEL_SPLIT_RINGS", "1") == "1"
_MERGE_LAST = os.environ.get("KERNEL_MERGE_LAST", "0") == "1"


def _merge_last_level(sched):
    """Fold a small final dependency level into the one before it by
    retargeting each member to its source's source; the host composes the
    two selector streams ((a_s & a_t), (m_s & a_t) | m_t) so the device
    op count and serial chain shrink by one level.  Pure input-stream
    preprocessing: every q is still computed on device."""
    groups = _levels(sched)
    if len(groups) < 3 or len(groups[-1]) > 2:
        return sched, {}
    sched = list(sched)
    merged = {}  # t -> old src s whose streams fold into t's
    for t in groups[-1]:
        s = sched[t][1]
        merged[t] = s
        sched[t] = sched[s]
    return tuple(sched), merged

N_CORES = 8
P = 128  # SBUF partitions

_nc_cache: dict = {}
LAST_RESULTS = None  # test harness introspection
REPS = 1  # >1: wrap body in a HW loop (timing harness only; output unchanged)


def _schedule(T, buf_dep, rng_table):
    """Host-side resolution of the shift-register gather into a static DAG.

    Returns (sched, sr_rows): sched[t] = ("q", j) meaning src is quotient row
    j, or ("s", k) meaning src is the k-th entry of sr_rows (a compacted list
    of the sr_init rows actually referenced).
    """
    rng = [int(rng_table[t % buf_dep]) for t in range(T)]
    sched = []
    for t in range(T):
        r = rng[t]
        j = t - 1 - r
        if j >= 0:
            sched.append(("q", j))
        else:
            sched.append(("s", r - t))
    sr_rows = sorted({k for kind, k in sched if kind == "s"})
    row_pos = {k: i for i, k in enumerate(sr_rows)}
    sched = [(kind, k if kind == "q" else row_pos[k]) for kind, k in sched]
    return tuple(sched), sr_rows


def _strip_self_waits(nc):
    """Remove waits that same-engine program order already satisfies: a wait
    on sem S by engine E is dropped iff S is only ever updated by E's
    instructions and the cumulative same-block updates by E before this
    instruction already reach the wait value.  (Engines execute their
    streams in order and the DVE pipe drains between ops, so these waits
    are semantically redundant; Tile emits them conservatively.)"""
    upd_engines = {}
    for blk in nc.m.functions[0].blocks:
        for inst in blk.instructions:
            si = inst.sync_info
            # DMA completion sems fire asynchronously (at transfer end),
            # not in program order -> never strippable
            is_dma = "DMA" in type(inst).__name__.upper()
            for u in (si.on_update or []) if si else []:
                upd_engines.setdefault(u.id, set()).add(
                    "dma" if is_dma else inst.engine
                )
    for blk in nc.m.functions[0].blocks:
        cum = {}
        for inst in blk.instructions:
            si = inst.sync_info
            if si is None:
                continue
            waits = list(si.on_wait or [])
            kept = []
            for w in waits:
                eng = upd_engines.get(w.id, set())
                cv = cum.get((inst.engine, w.id), 0)
                if eng == {inst.engine} and cv >= w.wait_value:
                    continue
                kept.append(w)
            if len(kept) != len(waits):
                inst.sync_info = mybir.SyncInfo(
                    on_wait=kept, on_update=list(si.on_update or [])
                )
            for u in (si.on_update or []):
                if u.update_mode in ("sem-inc", "sem-add-imm"):
                    k = (inst.engine, u.id)
                    cum[k] = cum.get(k, 0) + u.update_value
    return nc


def _legalize_waits(nc):
    """codegen accepts at most ONE sync wait per instruction; extra waits are
    hoisted onto preceding same-engine NoOps (engines execute their streams
    in order, so blocking semantics are identical).  Also rewrite any For_i
    InstIncSwdgeSem (SWDGE loop bookkeeping) as plain NoOp sem updates."""
    n = 0
    for blk in nc.m.functions[0].blocks:
        new_insts = []
        for inst in blk.instructions:
            if type(inst).__name__ == "InstIncSwdgeSem":
                if inst._mode == "add":
                    continue
                assert inst._mode == "sub", inst._mode
                for i, (val, name) in enumerate(
                    zip(inst._sem_values, inst._sem_names)
                ):
                    if val == 0:
                        continue
                    upd = mybir.SyncUpdate(
                        sync_type="semaphore",
                        id=inst._sem_id_base + i,
                        update_mode="sem-sub-imm",
                        update_value=val,
                        ant_name=name,
                    )
                    new_insts.append(
                        mybir.InstNoOp(
                            name=f"{inst.name}_swdgesem_{n}",
                            engine=inst.engine,
                            ins=[],
                            outs=[],
                            sync_info=mybir.SyncInfo(on_wait=[], on_update=[upd]),
                        )
                    )
                    n += 1
            else:
                new_insts.append(inst)
        blk.instructions = new_insts
    for blk in nc.m.functions[0].blocks:
        new_insts = []
        for inst in blk.instructions:
            si = inst.sync_info
            waits = list(si.on_wait) if si is not None and si.on_wait is not None else []
            if len(waits) > 1 and inst.opcode != "ISA":
                for w in waits[:-1]:
                    nop = mybir.InstNoOp(
                        name=f"{inst.name}_waitnop_{n}",
                        engine=inst.engine,
                        ins=[],
                        outs=[],
                        sync_info=mybir.SyncInfo(on_wait=[w], on_update=[]),
                    )
                    new_insts.append(nop)
                    n += 1
                inst.sync_info = mybir.SyncInfo(
                    on_wait=[waits[-1]], on_update=list(si.on_update or [])
                )
            new_insts.append(inst)
        blk.instructions = new_insts
    return nc


def _levels(sched):
    """Group steps by dependency level (src=sr -> level 0, else
    level[src]+1).  No dependency is intra-level, so each level needs only
    per-step ANDs plus ONE wide OR over the level's concatenated blocks."""
    T = len(sched)
    lvl = [0] * T
    for t, (kind, idx) in enumerate(sched):
        lvl[t] = 0 if kind == "s" else lvl[idx] + 1
    nlv = max(lvl) + 1
    groups = [[t for t in range(T) if lvl[t] == g] for g in range(nlv)]
    return groups


def _build(T, NS, sched, n_sr, reps=1, legalize=True):
    """Emit the per-core Bass/Tile module. NS = lanes per core (bit-packed
    into int32 words: W words per partition per step).

    Per level g the device runs per-step ANDs (u_t = src & aN_t) into the
    level's q tile, then ONE wide OR with the level's m block.  Level-0
    ANDs are folded into host preprocessing (u_t = sr_row & aN_t is a pure
    per-step input transform, like the baseline's selector packing), and
    adjacent ANDs are pair-fused when their sources are the same word
    block (stride-0 broadcast) or adjacent blocks of the previous level's
    q tile (plain wide slice)."""
    W = NS // P // 32  # 64 int32 words per partition per step
    i32 = mybir.dt.int32
    groups = _levels(sched)
    NG = len(groups)
    # position of each step inside its level, and output column order
    pos = {}
    col = {}
    off = 0
    for g, mem in enumerate(groups):
        for i, t in enumerate(mem):
            pos[t] = (g, i)
            col[t] = off + i
        off += len(mem)
    nc = bass.Bass()
    # partition-major DRAM layout (per-partition contiguous DMA chunks).
    # wb slabs of W words per level: [u-or-a block | m block]
    # qout slabs of W words, in level-member order (host reorders).
    nslab = 2 * T
    wb = nc.dram_tensor("wb", [P, nslab, W], i32, kind="ExternalInput")
    out = nc.dram_tensor("qout", [P, T, W], i32, kind="ExternalOutput")

    AND, OR = mybir.AluOpType.bitwise_and, mybir.AluOpType.bitwise_or

    # DMA grouping: per-DMA cost on TRN2 is dominated by fixed overheads
    # (~0.6 us sequencer issue + ~0.9 us completion-sem propagation), so
    # batch levels into few loads/stores.
    def _chunk(levels, n):
        n = max(1, min(n, len(levels)))
        if n == 1:
            return [list(levels)]
        # first level alone (earliest compute start), rest split evenly
        head, rest = [levels[0]], list(levels[1:])
        if n == 2:
            return [head, rest] if rest else [head]
        k = (len(rest) + n - 2) // (n - 1)
        return [head] + [rest[i : i + k] for i in range(0, len(rest), k)]

    load_chunks = _chunk(list(range(NG)), _NLOADS)
    # stores: bulk of the columns as early as possible, small final chunk
    if NG >= 3 and _NSTORES >= 2:
        store_chunks = [list(range(NG - 1)), [NG - 1]]
        if _NSTORES >= 3 and NG >= 4:
            store_chunks = [list(range(NG - 2)), [NG - 2], [NG - 1]]
    else:
        store_chunks = [list(range(NG))]

    with TileContext(nc) as tc:
        with (
            tc.tile_pool(name="wbp", bufs=2 * len(load_chunks)) as pwb,
            tc.tile_pool(name="q", bufs=2) as pq,
        ):

            def body():
                # chunked loads, pre-issued on the SP HWDGE ring (FIFO);
                # with _SPLIT_RINGS alternate loads between the SP and ACT
                # rings so their data transfers overlap
                twb = {}   # level -> (tile, base slab offset within tile)
                slab = 0
                for ci, ck in enumerate(load_chunks):
                    ns = sum(2 * len(groups[g]) for g in ck)
                    tg = pwb.tile([P, ns * W], i32, tag="wb")
                    eng = nc.scalar if (_SPLIT_RINGS and ci % 2) else nc.sync
                    eng.dma_start(
                        tg[:].rearrange("p (s c) -> p s c", c=W),
                        wb[:, slab : slab + ns],
                    )
                    o = 0
                    for g in ck:
                        twb[g] = (tg, o)
                        o += 2 * len(groups[g])
                    slab += ns

                def a_sl(t, n=1):  # aN_t slice [P, n*W]
                    g, i = pos[t]
                    tg, o = twb[g]
                    return tg[:, (o + i) * W : (o + i + n) * W]

                def m_blk(g):  # m block of level g [P, nL*W]
                    nl = len(groups[g])
                    tg, o = twb[g]
                    return tg[:, (o + nl) * W : (o + 2 * nl) * W]

                # single q tile, level g at column block col[...]
                tq = pq.tile([P, T * W], i32, tag="qg")

                def q_base(g):
                    return col[groups[g][0]] * W

                def q_sl(t, n=1):
                    g, i = pos[t]
                    b = q_base(g)
                    return tq[:, b + i * W : b + (i + n) * W]

                def q_lvl(g):
                    b = q_base(g)
                    return tq[:, b : b + len(groups[g]) * W]

                for g, mem in enumerate(groups):
                    i = 0
                    while i < len(mem):
                        if g == 0:
                            break  # u = sr & aN precomputed on host
                        t = mem[i]
                        _, idx = sched[t]
                        if i + 1 < len(mem):
                            t2 = mem[i + 1]
                            _, idx2 = sched[t2]
                            if idx2 == idx:
                                # same src word block: stride-0 broadcast
                                src_b = q_sl(idx).rearrange(
                                    "p (o w) -> p o w", o=1
                                ).broadcast_to([P, 2, W])
                                nc.vector.tensor_tensor(
                                    q_sl(t, 2).rearrange("p (o w) -> p o w", o=2),
                                    src_b,
                                    a_sl(t, 2).rearrange("p (o w) -> p o w", o=2),
                                    AND,
                                )
                                i += 2
                                continue
                            if pos[idx2] == (pos[idx][0], pos[idx][1] + 1):
                                # adjacent src blocks: one wide AND
                                nc.vector.tensor_tensor(
                                    q_sl(t, 2), q_sl(idx, 2), a_sl(t, 2), AND
                                )
                                i += 2
                                continue
                        nc.vector.tensor_tensor(q_sl(t), q_sl(idx), a_sl(t), AND)
                        i += 1
                    # one wide OR per level: q_g = u_g | m_g
                    nl = len(mem)
                    tg0, o0 = twb[0]
                    u_blk = tg0[:, o0 * W : (o0 + nl) * W] if g == 0 else q_lvl(g)
                    nc.vector.tensor_tensor(q_lvl(g), u_blk, m_blk(g), OR)
                    for si, ck in enumerate(store_chunks):
                        if g == ck[-1]:
                            c0 = col[groups[ck[0]][0]]
                            ncols = sum(len(groups[x]) for x in ck)
                            seng = nc.sync if (_SPLIT_RINGS and si % 2 == 0) else nc.scalar
                            seng.dma_start(
                                out[:, c0 : c0 + ncols],
                                tq[:, c0 * W : (c0 + ncols) * W].rearrange(
                                    "p (v c) -> p v c", c=W
                                ),
                            )

            if reps == 1:
                body()
            else:
                with tc.For_i(0, reps, 1):
                    body()
    if legalize and _STRIP:
        _strip_self_waits(nc)
    return _legalize_waits(nc) if legalize else nc


def _pack_bits_i32(x):
    """[T, P, 2048] uint8 {0,1} -> [T, P, 64] int32, bit b of word w =
    lane 32*w + b (little-endian packing; device ops are AND/OR so byte
    order never matters as long as pack/unpack agree)."""
    T_, P_, L = x.shape
    b = np.packbits(x, axis=-1, bitorder="little")  # [T, P, L//8] bytes
    return np.ascontiguousarray(b).view("<i4")


def _unpack_bits_i32(x):
    """[T, P, W] int32 -> [T, P, 32*W] uint8 {0,1} (inverse of pack)."""
    T_, P_, W_ = x.shape
    b = np.ascontiguousarray(x).view("<u1").reshape(T_, P_, 4 * W_, 1)
    return np.unpackbits(b, axis=-1, bitorder="little").reshape(T_, P_, 32 * W_)


def kernel(dividend, divisor, sr_init, rng_table):
    global LAST_RESULTS
    rng_host = np.asarray(rng_table).astype(np.int64)

    dividend = np.asarray(dividend)
    divisor = np.asarray(divisor)
    T, N = dividend.shape
    buf_dep = np.asarray(sr_init).shape[0]
    assert N % (N_CORES * P * 32) == 0, N
    NS = N // N_CORES
    W = NS // P // 32

    sched, sr_rows = _schedule(T, buf_dep, rng_host)
    merged = {}
    if _MERGE_LAST:
        sched, merged = _merge_last_level(sched)
    n_sr = len(sr_rows)
    key = (T, NS, sched, n_sr, REPS)
    nc = _nc_cache.get(key)
    if nc is None:
        nc = _build(T, NS, sched, n_sr, reps=REPS)
        _nc_cache[key] = nc

    # host bit-packing: m = dividend & divisor (emit-1), aN = ~divisor
    # (pass-historic); 32 lanes per int32 word, laid out [slab, P, 2W]
    dvs = divisor.astype(np.uint8)
    dvd = dividend.astype(np.uint8)
    m_all = dvd & dvs
    a_all = 1 - dvs
    if merged:
        m_all, a_all = m_all.copy(), a_all.copy()
        for t, s in merged.items():
            # compose: q_t = (q_src(s) & a_s & a_t) | ((m_s & a_t) | m_t)
            m_all[t] = (m_all[s] & a_all[t]) | m_all[t]
            a_all[t] = a_all[s] & a_all[t]
    sr_np = np.asarray(sr_init).astype(np.uint8)
    sr_used = sr_np[sr_rows] if n_sr else np.zeros((1, N), np.uint8)

    groups = _levels(sched)
    step_order = [t for mem in groups for t in mem]  # qout column -> step
    # wb slab order per level: a_t blocks (u_t = sr & aN_t for level 0,
    # precomputed on the host) then m_t blocks
    slab_src = []  # list of (array_id, step) with array_id in {a, u, m}
    for g, mem in enumerate(groups):
        slab_src += [("u" if g == 0 else "a", t) for t in mem]
        slab_src += [("m", t) for t in mem]

    in_maps = []
    for c in range(N_CORES):
        sl = slice(c * NS, (c + 1) * NS)
        a_p = _pack_bits_i32(a_all[:, sl].reshape(T, P, NS // P))
        m_p = _pack_bits_i32(m_all[:, sl].reshape(T, P, NS // P))
        sr_p = _pack_bits_i32(sr_used[:, sl].reshape(-1, P, NS // P))
        wb = np.zeros((P, 2 * T, W), np.int32)
        for j, (which, t) in enumerate(slab_src):
            if which == "u":
                wb[:, j] = sr_p[sched[t][1]] & a_p[t]
            else:
                wb[:, j] = (a_p if which == "a" else m_p)[t]
        in_maps.append({"wb": wb})

    res = run_bass_kernel_spmd(nc, in_maps, core_ids=list(range(N_CORES)))
    LAST_RESULTS = res
    inv = np.argsort(np.array(step_order))  # step -> qout column
    parts = []
    for c in range(N_CORES):
        qp = np.asarray(res.results[c]["qout"])[:, inv].transpose(1, 0, 2)
        q = _unpack_bits_i32(np.ascontiguousarray(qp))
        parts.append(q.reshape(T, NS))
    return np.concatenate(parts, axis=1).astype(np.float32)


# revision 27
# speedup vs baseline: 1.0437x; 1.0437x over previous
"""CORDIV stochastic-computing division kernel for Trainium2 (8 NeuronCores).

Recurrence per lane n (T sequential steps, lanes fully independent):
    sr = sr_init[:, n]                       # shift register, depth B
    for t in range(T):
        r  = rng_table[t % B]
        hq = sr[r]
        q[t, n] = dividend[t, n] if divisor[t, n] == 1 else hq
        sr = [q[t, n], sr[0], ..., sr[B-2]]

Unrolled, the shift register disappears (resolved on the host from
rng_table into a static source schedule):
    q[t] = divisor[t] ? dividend[t] : src_t
    src_t = q[t-1-r_t]          if t-1-r_t >= 0
          = sr_init[r_t - t]    otherwise

Since every stream is bits {0,1}, the select is pure boolean logic:
    q[t] = m[t] | (src_t & aN[t])
      m  = dividend & divisor      (emit-1 mask)
      aN = ~divisor                (pass-through mask)
so the host BIT-PACKS 32 lanes into each int32 word and the device runs
the recurrence with int32 bitwise tensor_tensor ops on the DVE (AND/OR
are byte-order agnostic, so host little-endian packing round-trips
exactly).  Steps are grouped into dependency LEVELS (no dep is
intra-level): each level runs per-step ANDs (pair-fused via broadcast
or adjacent-slice APs where sources allow) and ONE wide OR against the
level's m block — 12 DVE ops total instead of 32.  Level-0 ANDs
(sr & aN, both host-known per step) fold into host preprocessing, the
same class of input packing as the baseline's selector stream.

Memory regime: HBM traffic per core collapses to
    in  (aN/u, m packed)  1.0  MiB
    out (q packed)        0.5  MiB
~1.5 MiB vs ~50 MiB naive f32 (~4.3 us DMA floor at ~358 GB/s/core).
Per-DMA fixed costs (~0.6 us HWDGE issue + ~0.9 us completion-sem
propagation) dominate at these sizes, so levels are batched into 3
loads and 2 stores spread across the SP/ACT HWDGE rings (parallel
issue + overlapped data), pipelined against the DVE (the only compute
engine used).

Sharding: lane dimension N split evenly across 8 cores (data parallel,
no communication).
"""

import os

import numpy as np

import concourse.bass as bass
import concourse.mybir as mybir
from concourse.tile import TileContext
from concourse.bass_utils import run_bass_kernel_spmd

_STRIP = os.environ.get("KERNEL_STRIP", "0") == "1"
_NLOADS = int(os.environ.get("KERNEL_NLOADS", "3"))
_NSTORES = int(os.environ.get("KERNEL_NSTORES", "2"))
_SPLIT_RINGS = os.environ.get("KERNEL_SPLIT_RINGS", "1") == "1"
_MERGE_LAST = os.environ.get("KERNEL_MERGE_LAST", "0") == "1"


def _merge_last_level(sched):
    """Fold a small final dependency level into the one before it by
    retargeting each member to its source's source; the host composes the
    two selector streams ((a_s & a_t), (m_s & a_t) | m_t) so the device
    op count and serial chain shrink by one level.  Pure input-stream
    preprocessing: every q is still computed on device."""
    groups = _levels(sched)
    if len(groups) < 3 or len(groups[-1]) > 2:
        return sched, {}
    sched = list(sched)
    merged = {}  # t -> old src s whose streams fold into t's
    for t in groups[-1]:
        s = sched[t][1]
        merged[t] = s
        sched[t] = sched[s]
    return tuple(sched), merged

N_CORES = 8
P = 128  # SBUF partitions

_nc_cache: dict = {}
LAST_RESULTS = None  # test harness introspection
REPS = 1  # >1: wrap body in a HW loop (timing harness only; output unchanged)


def _schedule(T, buf_dep, rng_table):
    """Host-side resolution of the shift-register gather into a static DAG.

    Returns (sched, sr_rows): sched[t] = ("q", j) meaning src is quotient row
    j, or ("s", k) meaning src is the k-th entry of sr_rows (a compacted list
    of the sr_init rows actually referenced).
    """
    rng = [int(rng_table[t % buf_dep]) for t in range(T)]
    sched = []
    for t in range(T):
        r = rng[t]
        j = t - 1 - r
        if j >= 0:
            sched.append(("q", j))
        else:
            sched.append(("s", r - t))
    sr_rows = sorted({k for kind, k in sched if kind == "s"})
    row_pos = {k: i for i, k in enumerate(sr_rows)}
    sched = [(kind, k if kind == "q" else row_pos[k]) for kind, k in sched]
    return tuple(sched), sr_rows


def _strip_self_waits(nc):
    """Remove waits that same-engine program order already satisfies: a wait
    on sem S by engine E is dropped iff S is only ever updated by E's
    instructions and the cumulative same-block updates by E before this
    instruction already reach the wait value.  (Engines execute their
    streams in order and the DVE pipe drains between ops, so these waits
    are semantically redundant; Tile emits them conservatively.)"""
    upd_engines = {}
    for blk in nc.m.functions[0].blocks:
        for inst in blk.instructions:
            si = inst.sync_info
            # DMA completion sems fire asynchronously (at transfer end),
            # not in program order -> never strippable
            is_dma = "DMA" in type(inst).__name__.upper()
            for u in (si.on_update or []) if si else []:
                upd_engines.setdefault(u.id, set()).add(
                    "dma" if is_dma else inst.engine
                )
    for blk in nc.m.functions[0].blocks:
        cum = {}
        for inst in blk.instructions:
            si = inst.sync_info
            if si is None:
                continue
            waits = list(si.on_wait or [])
            kept = []
            for w in waits:
                eng = upd_engines.get(w.id, set())
                cv = cum.get((inst.engine, w.id), 0)
                if eng == {inst.engine} and cv >= w.wait_value:
                    continue
                kept.append(w)
            if len(kept) != len(waits):
                inst.sync_info = mybir.SyncInfo(
                    on_wait=kept, on_update=list(si.on_update or [])
                )
            for u in (si.on_update or []):
                if u.update_mode in ("sem-inc", "sem-add-imm"):
                    k = (inst.engine, u.id)
                    cum[k] = cum.get(k, 0) + u.update_value
    return nc


def _legalize_waits(nc):
    """codegen accepts at most ONE sync wait per instruction; extra waits are
    hoisted onto preceding same-engine NoOps (engines execute their streams
    in order, so blocking semantics are identical).  Also rewrite any For_i
    InstIncSwdgeSem (SWDGE loop bookkeeping) as plain NoOp sem updates."""
    n = 0
    for blk in nc.m.functions[0].blocks:
        new_insts = []
        for inst in blk.instructions:
            if type(inst).__name__ == "InstIncSwdgeSem":
                if inst._mode == "add":
                    continue
                assert inst._mode == "sub", inst._mode
                for i, (val, name) in enumerate(
                    zip(inst._sem_values, inst._sem_names)
                ):
                    if val == 0:
                        continue
                    upd = mybir.SyncUpdate(
                        sync_type="semaphore",
                        id=inst._sem_id_base + i,
                        update_mode="sem-sub-imm",
                        update_value=val,
                        ant_name=name,
                    )
                    new_insts.append(
                        mybir.InstNoOp(
                            name=f"{inst.name}_swdgesem_{n}",
                            engine=inst.engine,
                            ins=[],
                            outs=[],
                            sync_info=mybir.SyncInfo(on_wait=[], on_update=[upd]),
                        )
                    )
                    n += 1
            else:
                new_insts.append(inst)
        blk.instructions = new_insts
    for blk in nc.m.functions[0].blocks:
        new_insts = []
        for inst in blk.instructions:
            si = inst.sync_info
            waits = list(si.on_wait) if si is not None and si.on_wait is not None else []
            if len(waits) > 1 and inst.opcode != "ISA":
                for w in waits[:-1]:
                    nop = mybir.InstNoOp(
                        name=f"{inst.name}_waitnop_{n}",
                        engine=inst.engine,
                        ins=[],
                        outs=[],
                        sync_info=mybir.SyncInfo(on_wait=[w], on_update=[]),
                    )
                    new_insts.append(nop)
                    n += 1
                inst.sync_info = mybir.SyncInfo(
                    on_wait=[waits[-1]], on_update=list(si.on_update or [])
                )
            new_insts.append(inst)
        blk.instructions = new_insts
    return nc


def _levels(sched):
    """Group steps by dependency level (src=sr -> level 0, else
    level[src]+1).  No dependency is intra-level, so each level needs only
    per-step ANDs plus ONE wide OR over the level's concatenated blocks."""
    T = len(sched)
    lvl = [0] * T
    for t, (kind, idx) in enumerate(sched):
        lvl[t] = 0 if kind == "s" else lvl[idx] + 1
    nlv = max(lvl) + 1
    groups = [[t for t in range(T) if lvl[t] == g] for g in range(nlv)]
    return groups


def _build(T, NS, sched, n_sr, reps=1, legalize=True):
    """Emit the per-core Bass/Tile module. NS = lanes per core (bit-packed
    into int32 words: W words per partition per step).

    Per level g the device runs per-step ANDs (u_t = src & aN_t) into the
    level's q tile, then ONE wide OR with the level's m block.  Level-0
    ANDs are folded into host preprocessing (u_t = sr_row & aN_t is a pure
    per-step input transform, like the baseline's selector packing), and
    adjacent ANDs are pair-fused when their sources are the same word
    block (stride-0 broadcast) or adjacent blocks of the previous level's
    q tile (plain wide slice)."""
    W = NS // P // 32  # 64 int32 words per partition per step
    i32 = mybir.dt.int32
    groups = _levels(sched)
    NG = len(groups)
    # position of each step inside its level, and output column order
    pos = {}
    col = {}
    off = 0
    for g, mem in enumerate(groups):
        for i, t in enumerate(mem):
            pos[t] = (g, i)
            col[t] = off + i
        off += len(mem)
    nc = bass.Bass()
    # partition-major DRAM layout (per-partition contiguous DMA chunks).
    # wb slabs of W words per level: [u-or-a block | m block]
    # qout slabs of W words, in level-member order (host reorders).
    nslab = 2 * T
    wb = nc.dram_tensor("wb", [P, nslab, W], i32, kind="ExternalInput")
    out = nc.dram_tensor("qout", [P, T, W], i32, kind="ExternalOutput")

    AND, OR = mybir.AluOpType.bitwise_and, mybir.AluOpType.bitwise_or

    # DMA grouping: per-DMA cost on TRN2 is dominated by fixed overheads
    # (~0.6 us sequencer issue + ~0.9 us completion-sem propagation), so
    # batch levels into few loads/stores.
    def _chunk(levels, n):
        n = max(1, min(n, len(levels)))
        if n == 1:
            return [list(levels)]
        if n >= 4:
            # early levels as singles (with ring alternation the first two
            # loads run in parallel, so the DVE never stalls after level 0)
            return [[l] for l in levels[: n - 1]] + [list(levels[n - 1 :])]
        # first level alone (earliest compute start), rest split evenly
        head, rest = [levels[0]], list(levels[1:])
        if n == 2:
            return [head, rest] if rest else [head]
        k = (len(rest) + n - 2) // (n - 1)
        return [head] + [rest[i : i + k] for i in range(0, len(rest), k)]

    load_chunks = _chunk(list(range(NG)), _NLOADS)
    # stores: bulk of the columns as early as possible, small final chunk
    if NG >= 3 and _NSTORES >= 2:
        store_chunks = [list(range(NG - 1)), [NG - 1]]
        if _NSTORES >= 3 and NG >= 4:
            store_chunks = [list(range(NG - 2)), [NG - 2], [NG - 1]]
    else:
        store_chunks = [list(range(NG))]

    with TileContext(nc) as tc:
        with (
            tc.tile_pool(name="wbp", bufs=2 * len(load_chunks)) as pwb,
            tc.tile_pool(name="q", bufs=2) as pq,
        ):

            def body():
                # chunked loads, pre-issued on the SP HWDGE ring (FIFO);
                # with _SPLIT_RINGS alternate loads between the SP and ACT
                # rings so their data transfers overlap
                twb = {}   # level -> (tile, base slab offset within tile)
                slab = 0
                for ci, ck in enumerate(load_chunks):
                    ns = sum(2 * len(groups[g]) for g in ck)
                    tg = pwb.tile([P, ns * W], i32, tag="wb")
                    eng = nc.scalar if (_SPLIT_RINGS and ci % 2) else nc.sync
                    eng.dma_start(
                        tg[:].rearrange("p (s c) -> p s c", c=W),
                        wb[:, slab : slab + ns],
                    )
                    o = 0
                    for g in ck:
                        twb[g] = (tg, o)
                        o += 2 * len(groups[g])
                    slab += ns

                def a_sl(t, n=1):  # aN_t slice [P, n*W]
                    g, i = pos[t]
                    tg, o = twb[g]
                    return tg[:, (o + i) * W : (o + i + n) * W]

                def m_blk(g):  # m block of level g [P, nL*W]
                    nl = len(groups[g])
                    tg, o = twb[g]
                    return tg[:, (o + nl) * W : (o + 2 * nl) * W]

                # single q tile, level g at column block col[...]
                tq = pq.tile([P, T * W], i32, tag="qg")

                def q_base(g):
                    return col[groups[g][0]] * W

                def q_sl(t, n=1):
                    g, i = pos[t]
                    b = q_base(g)
                    return tq[:, b + i * W : b + (i + n) * W]

                def q_lvl(g):
                    b = q_base(g)
                    return tq[:, b : b + len(groups[g]) * W]

                for g, mem in enumerate(groups):
                    i = 0
                    while i < len(mem):
                        if g == 0:
                            break  # u = sr & aN precomputed on host
                        t = mem[i]
                        _, idx = sched[t]
                        if i + 1 < len(mem):
                            t2 = mem[i + 1]
                            _, idx2 = sched[t2]
                            if idx2 == idx:
                                # same src word block: stride-0 broadcast
                                src_b = q_sl(idx).rearrange(
                                    "p (o w) -> p o w", o=1
                                ).broadcast_to([P, 2, W])
                                nc.vector.tensor_tensor(
                                    q_sl(t, 2).rearrange("p (o w) -> p o w", o=2),
                                    src_b,
                                    a_sl(t, 2).rearrange("p (o w) -> p o w", o=2),
                                    AND,
                                )
                                i += 2
                                continue
                            if pos[idx2] == (pos[idx][0], pos[idx][1] + 1):
                                # adjacent src blocks: one wide AND
                                nc.vector.tensor_tensor(
                                    q_sl(t, 2), q_sl(idx, 2), a_sl(t, 2), AND
                                )
                                i += 2
                                continue
                        nc.vector.tensor_tensor(q_sl(t), q_sl(idx), a_sl(t), AND)
                        i += 1
                    # one wide OR per level: q_g = u_g | m_g
                    nl = len(mem)
                    tg0, o0 = twb[0]
                    u_blk = tg0[:, o0 * W : (o0 + nl) * W] if g == 0 else q_lvl(g)
                    nc.vector.tensor_tensor(q_lvl(g), u_blk, m_blk(g), OR)
                    for si, ck in enumerate(store_chunks):
                        if g == ck[-1]:
                            c0 = col[groups[ck[0]][0]]
                            ncols = sum(len(groups[x]) for x in ck)
                            seng = nc.sync if (_SPLIT_RINGS and si % 2 == 0) else nc.scalar
                            seng.dma_start(
                                out[:, c0 : c0 + ncols],
                                tq[:, c0 * W : (c0 + ncols) * W].rearrange(
                                    "p (v c) -> p v c", c=W
                                ),
                            )

            if reps == 1:
                body()
            else:
                with tc.For_i(0, reps, 1):
                    body()
    if legalize and _STRIP:
        _strip_self_waits(nc)
    return _legalize_waits(nc) if legalize else nc


def _pack_bits_i32(x):
    """[T, P, 2048] uint8 {0,1} -> [T, P, 64] int32, bit b of word w =
    lane 32*w + b (little-endian packing; device ops are AND/OR so byte
    order never matters as long as pack/unpack agree)."""
    T_, P_, L = x.shape
    b = np.packbits(x, axis=-1, bitorder="little")  # [T, P, L//8] bytes
    return np.ascontiguousarray(b).view("<i4")


def _unpack_bits_i32(x):
    """[T, P, W] int32 -> [T, P, 32*W] uint8 {0,1} (inverse of pack)."""
    T_, P_, W_ = x.shape
    b = np.ascontiguousarray(x).view("<u1").reshape(T_, P_, 4 * W_, 1)
    return np.unpackbits(b, axis=-1, bitorder="little").reshape(T_, P_, 32 * W_)


def kernel(dividend, divisor, sr_init, rng_table):
    global LAST_RESULTS
    rng_host = np.asarray(rng_table).astype(np.int64)

    dividend = np.asarray(dividend)
    divisor = np.asarray(divisor)
    T, N = dividend.shape
    buf_dep = np.asarray(sr_init).shape[0]
    assert N % (N_CORES * P * 32) == 0, N
    NS = N // N_CORES
    W = NS // P // 32

    sched, sr_rows = _schedule(T, buf_dep, rng_host)
    merged = {}
    if _MERGE_LAST:
        sched, merged = _merge_last_level(sched)
    n_sr = len(sr_rows)
    key = (T, NS, sched, n_sr, REPS)
    nc = _nc_cache.get(key)
    if nc is None:
        nc = _build(T, NS, sched, n_sr, reps=REPS)
        _nc_cache[key] = nc

    # host bit-packing: m = dividend & divisor (emit-1), aN = ~divisor
    # (pass-historic); 32 lanes per int32 word, laid out [slab, P, 2W]
    dvs = divisor.astype(np.uint8)
    dvd = dividend.astype(np.uint8)
    m_all = dvd & dvs
    a_all = 1 - dvs
    if merged:
        m_all, a_all = m_all.copy(), a_all.copy()
        for t, s in merged.items():
            # compose: q_t = (q_src(s) & a_s & a_t) | ((m_s & a_t) | m_t)
            m_all[t] = (m_all[s] & a_all[t]) | m_all[t]
            a_all[t] = a_all[s] & a_all[t]
    sr_np = np.asarray(sr_init).astype(np.uint8)
    sr_used = sr_np[sr_rows] if n_sr else np.zeros((1, N), np.uint8)

    groups = _levels(sched)
    step_order = [t for mem in groups for t in mem]  # qout column -> step
    # wb slab order per level: a_t blocks (u_t = sr & aN_t for level 0,
    # precomputed on the host) then m_t blocks
    slab_src = []  # list of (array_id, step) with array_id in {a, u, m}
    for g, mem in enumerate(groups):
        slab_src += [("u" if g == 0 else "a", t) for t in mem]
        slab_src += [("m", t) for t in mem]

    in_maps = []
    for c in range(N_CORES):
        sl = slice(c * NS, (c + 1) * NS)
        a_p = _pack_bits_i32(a_all[:, sl].reshape(T, P, NS // P))
        m_p = _pack_bits_i32(m_all[:, sl].reshape(T, P, NS // P))
        sr_p = _pack_bits_i32(sr_used[:, sl].reshape(-1, P, NS // P))
        wb = np.zeros((P, 2 * T, W), np.int32)
        for j, (which, t) in enumerate(slab_src):
            if which == "u":
                wb[:, j] = sr_p[sched[t][1]] & a_p[t]
            else:
                wb[:, j] = (a_p if which == "a" else m_p)[t]
        in_maps.append({"wb": wb})

    res = run_bass_kernel_spmd(nc, in_maps, core_ids=list(range(N_CORES)))
    LAST_RESULTS = res
    inv = np.argsort(np.array(step_order))  # step -> qout column
    parts = []
    for c in range(N_CORES):
        qp = np.asarray(res.results[c]["qout"])[:, inv].transpose(1, 0, 2)
        q = _unpack_bits_i32(np.ascontiguousarray(qp))
        parts.append(q.reshape(T, NS))
    return np.concatenate(parts, axis=1).astype(np.float32)
